# revision 5
# baseline (speedup 1.0000x reference)
"""Trainium2 Bass kernel for the Actor MLP (BatchNorm -> 3-layer MLP -> atan2).

Data-parallel across 8 NeuronCores: batch sharded 8192 rows/core, weights
replicated. BatchNorm batch stats via per-shard bn_stats + 8KB AllReduce.

Matmuls run in float32r (fp32 with the mantissa rounded to ~11 bits): on
TRN2 this streams at the same 1 cycle/row as bf16 but with only ~2^-12
operand rounding, which keeps the atan2 branch-cut (ty ~ 0, tx < 0) sign
flips low (~294 of 8.4M elements, rel err 1.98e-2 vs bf16's 7.7e-2).
Weights are pre-rounded to f32r granularity on the host and DMA'd directly
into f32r tiles; activations are rounded by the ACT engine on PSUM
eviction.

Batch is processed in 1024-column super-tiles (two 512-col halves sharing
each streamed W1/W2 m-slice, halving weight HBM traffic vs per-512
streaming). Layer 3 (only 256 outputs) is fused into the layer-2 m-loop:
each h2 m-slice feeds two accumulating matmuls into resident zy/zx PSUM
banks, so h2 never materializes in SBUF and the freed space pays for the
1024-wide h1. The L3 matmuls for m-slice m-1 are issued after the L2
k-loops of m so the in-order PE queue never waits on an ACT eviction.
All activations stay transposed [feature, batch]; x is staged via 4MB
DMAs into slots aliased (by pool tag) onto the h1/xt buffers, which are
dead during the stats pass.
"""

import numpy as np

P = 128
B_CORE = 8192            # batch rows per core
SUP = 1024               # super-tile batch columns (2 halves of 512)
NSUP = B_CORE // SUP     # 8
HALF = 512
NJ = SUP // P            # 8 natural [128, D_IN] blocks per super-tile
D_IN = 1024
K_IN = D_IN // P         # 8
D_H = 2048
K_H = D_H // P           # 16
D_ACT = 128
BN_EPS = 1e-5
N_CORES = 8
INV_PI = float(1.0 / np.pi)

_CACHE = {}

LAST_EXEC_NS = None
LAST_TRACE_DIR = None


def _build_nc():
    import concourse.mybir as mybir
    import concourse.tile as tile
    from concourse import bacc
    from concourse.masks import make_identity

    f32 = mybir.dt.float32
    f32r = mybir.dt.float32r
    AF = mybir.ActivationFunctionType
    ALU = mybir.AluOpType

    nc = bacc.Bacc()

    x_ext = nc.declare_dram_parameter("x", [B_CORE, D_IN], f32, isOutput=False)
    # weights pre-tiled on host: [m_slice, partition, k*128] so each m-slice
    # DMA reads one fully-contiguous block per partition
    w1t_ext = nc.declare_dram_parameter("w1t", [K_H, P, D_IN], f32r, isOutput=False)
    w2t_ext = nc.declare_dram_parameter("w2t", [K_H, P, D_H], f32r, isOutput=False)
    w3yt_ext = nc.declare_dram_parameter("w3yt", [D_H, D_ACT], f32r, isOutput=False)
    w3xt_ext = nc.declare_dram_parameter("w3xt", [D_H, D_ACT], f32r, isOutput=False)
    b1_ext = nc.declare_dram_parameter("b1r", [P, K_H], f32, isOutput=False)
    b2_ext = nc.declare_dram_parameter("b2r", [P, K_H], f32, isOutput=False)
    b3y_ext = nc.declare_dram_parameter("b3y", [P, 1], f32, isOutput=False)
    b3x_ext = nc.declare_dram_parameter("b3x", [P, 1], f32, isOutput=False)
    bnw_ext = nc.declare_dram_parameter("bnw", [P, K_IN], f32, isOutput=False)
    bnb_ext = nc.declare_dram_parameter("bnb", [P, K_IN], f32, isOutput=False)
    out_ext = nc.declare_dram_parameter("out", [B_CORE, D_ACT], f32, isOutput=True)

    with tile.TileContext(nc) as tc:
        with (
            tc.tile_pool(name="singles", bufs=1) as singles,
            tc.tile_pool(name="small", bufs=1) as small,
            # tags h1 (64KB) / xt (32KB); the stats pass stages raw x in the
            # same slots (they're dead until the MLP pass)
            tc.tile_pool(name="acts", bufs=1) as acts,
            tc.tile_pool(name="xn", bufs=4) as xn_pool,
            tc.tile_pool(name="w1m", bufs=3) as w1_pool,
            tc.tile_pool(name="w2m", bufs=3) as w2_pool,
            tc.tile_pool(name="h2m", bufs=4) as h2_pool,
            tc.tile_pool(name="epi", bufs=1) as epi_pool,
            tc.tile_pool(name="res", bufs=2) as res_pool,
            tc.tile_pool(name="xps", bufs=2, space="PSUM") as xps_pool,
            tc.tile_pool(name="mm", bufs=2, space="PSUM") as mm_pool,
            tc.tile_pool(name="zz", bufs=1, space="PSUM") as zz_pool,
            tc.tile_pool(name="dram", bufs=1, space="DRAM") as dram_pool,
        ):
            # ---- constants ----
            ident = singles.tile([P, P], f32)
            make_identity(nc, ident)

            bnws = singles.tile([P, K_IN], f32)
            nc.sync.dma_start(out=bnws, in_=bnw_ext[:])
            bnbs = singles.tile([P, K_IN], f32)
            nc.sync.dma_start(out=bnbs, in_=bnb_ext[:])

            # ---- pass 1: batch stats (4MB staged loads -> transpose -> bn_stats) ----
            stats = small.tile([P, K_IN, 2 * NSUP, 6], f32)
            mv = small.tile([P, K_IN, 2], f32)
            # pk = [mean, E[x^2]] / N_CORES, packed for the AllReduce
            pk = small.tile([P, K_IN, 2], f32)
            for s in range(NSUP):
                stage = acts.tile(
                    [P, NJ, D_IN], f32,
                    tag=("h1" if s % 2 == 0 else "xt"), name=f"stage{s}",
                )
                nc.gpsimd.dma_start(
                    out=stage,
                    in_=x_ext[s * SUP : (s + 1) * SUP, :].rearrange(
                        "(j p) c -> p j c", p=P
                    ),
                )
                for k in range(K_IN):
                    for h in range(2):
                        ps = xps_pool.tile([P, HALF], f32, tag="xps", name="ps")
                        for jj in range(4):
                            j = h * 4 + jj
                            nc.tensor.transpose(
                                ps[:, jj * P : (jj + 1) * P],
                                stage[:, j, k * P : (k + 1) * P],
                                ident,
                            )
                        nc.vector.bn_stats(out=stats[:, k, s * 2 + h, :], in_=ps)
                    if s == NSUP - 1:
                        # aggregate per k as soon as its last bn_stats lands
                        nc.vector.bn_aggr(out=mv[:, k, :], in_=stats[:, k, :, :])

            # MLP-pass constants, loaded behind the stats pass so the X stages
            # get the DMA queues at t=0
            w3ys = singles.tile([P, K_H, D_ACT], f32r)
            nc.sync.dma_start(out=w3ys, in_=w3yt_ext[:].rearrange("(k p) a -> p k a", p=P))
            w3xs = singles.tile([P, K_H, D_ACT], f32r)
            nc.sync.dma_start(out=w3xs, in_=w3xt_ext[:].rearrange("(k p) a -> p k a", p=P))
            b1s = singles.tile([P, K_H], f32)
            nc.sync.dma_start(out=b1s, in_=b1_ext[:])
            b2s = singles.tile([P, K_H], f32)
            nc.sync.dma_start(out=b2s, in_=b2_ext[:])
            b3ys = singles.tile([P, 1], f32)
            nc.sync.dma_start(out=b3ys, in_=b3y_ext[:])
            b3xs = singles.tile([P, 1], f32)
            nc.sync.dma_start(out=b3xs, in_=b3x_ext[:])

            # pack [mean, E[x^2]]/N_CORES for the AllReduce
            nc.vector.tensor_scalar_mul(pk[:, :, 0], mv[:, :, 0], 1.0 / N_CORES)
            nc.vector.tensor_mul(pk[:, :, 1], mv[:, :, 0], mv[:, :, 0])
            nc.vector.tensor_add(pk[:, :, 1], pk[:, :, 1], mv[:, :, 1])
            nc.vector.tensor_scalar_mul(pk[:, :, 1], pk[:, :, 1], 1.0 / N_CORES)

            cc_in = dram_pool.tile([P, K_IN, 2], f32)
            cc_out = dram_pool.tile([P, K_IN, 2], f32)
            # gpsimd's queue, so this 8KB doesn't sit behind weight prefetch MBs
            nc.gpsimd.dma_start(out=cc_in, in_=pk)
            nc.gpsimd.collective_compute(
                "AllReduce",
                ALU.add,
                replica_groups=[list(range(N_CORES))],
                ins=[cc_in.opt()],
                outs=[cc_out.opt()],
            )
            g = small.tile([P, K_IN, 2], f32)
            nc.gpsimd.dma_start(out=g, in_=cc_out)

            # global mean / var -> per-feature scale & shift
            gm = g[:, :, 0]
            var = small.tile([P, K_IN], f32)
            nc.vector.tensor_copy(out=var, in_=g[:, :, 1])
            gm2 = small.tile([P, K_IN], f32)
            nc.vector.tensor_mul(gm2, gm, gm)
            nc.vector.tensor_sub(var, var, gm2)
            eps_t = small.tile([P, 1], f32)
            nc.vector.memset(eps_t, BN_EPS)
            sq = small.tile([P, K_IN], f32)
            nc.scalar.activation(out=sq, in_=var, func=AF.Sqrt, bias=eps_t, scale=1.0)
            rstd = small.tile([P, K_IN], f32)
            nc.vector.reciprocal(out=rstd, in_=sq)
            scale = small.tile([P, K_IN], f32)
            nc.vector.tensor_mul(scale, bnws, rstd)
            shift = small.tile([P, K_IN], f32)
            nc.vector.tensor_mul(shift, gm, scale)
            nc.vector.tensor_sub(shift, bnbs, shift)

            # ---- pass 2: normalize + 3-layer MLP + atan2 epilogue ----
            pend = None  # deferred output-transpose work: (s, [rx_h0, rx_h1])

            def emit_out(pd):
                ps_, rxs = pd
                for h in range(2):
                    rp = xps_pool.tile([P, 4, P], f32, tag="xps", name="rp")
                    for jj in range(4):
                        nc.tensor.transpose(
                            rp[:, jj, :], rxs[h][:, jj * P : (jj + 1) * P], ident
                        )
                    rn = res_pool.tile([P, 4, P], f32, tag="rn", name="rn")
                    nc.scalar.activation(out=rn, in_=rp, func=AF.Copy)
                    r0 = ps_ * SUP + h * HALF
                    nc.sync.dma_start(
                        out=out_ext[r0 : r0 + HALF, :].rearrange(
                            "(j p) a -> p j a", p=P
                        ),
                        in_=rn,
                    )

            for s in range(NSUP):
                # stage natural x (4 x 1MB DMAs)
                xn = []
                for jb in range(4):
                    xnt = xn_pool.tile([P, 2, D_IN], f32, tag="xn", name=f"xn{s}_{jb}")
                    r0 = s * SUP + jb * 256
                    nc.gpsimd.dma_start(
                        out=xnt,
                        in_=x_ext[r0 : r0 + 256, :].rearrange("(j p) c -> p j c", p=P),
                    )
                    xn.append(xnt)

                # phase A: transpose + fused (x*scale + shift) normalize -> f32r
                xt = acts.tile([P, K_IN, 2, HALF], f32r, tag="xt", name=f"xt{s}")
                for h in range(2):
                    for k in range(K_IN):
                        ps = xps_pool.tile([P, HALF], f32, tag="xps", name="ps")
                        for jj in range(4):
                            j = h * 4 + jj
                            nc.tensor.transpose(
                                ps[:, jj * P : (jj + 1) * P],
                                xn[j // 2][:, j % 2, k * P : (k + 1) * P],
                                ident,
                            )
                        nc.scalar.activation(
                            out=xt[:, k, h, :],
                            in_=ps,
                            func=AF.Identity,
                            bias=shift[:, k : k + 1],
                            scale=scale[:, k : k + 1],
                        )

                # phase B (layer 1): h1T = relu(W1 @ xnT + b1); W1 m-slice
                # streamed once, used by both halves
                h1 = acts.tile([P, K_H, 2, HALF], f32r, tag="h1", name=f"h1_{s}")
                for m in range(K_H):
                    w1m = w1_pool.tile([P, K_IN, P], f32r, tag="w1m", name=f"w1m{s}_{m}")
                    nc.sync.dma_start(
                        out=w1m,
                        in_=w1t_ext[m].rearrange("p (k c) -> p k c", k=K_IN),
                    )
                    for h in range(2):
                        acc = mm_pool.tile([P, HALF], f32, tag="mm", name="acc")
                        for k in range(K_IN):
                            nc.tensor.matmul(
                                acc,
                                w1m[:, k, :],
                                xt[:, k, h, :],
                                start=(k == 0),
                                stop=(k == K_IN - 1),
                            )
                        nc.scalar.activation(
                            out=h1[:, m, h, :],
                            in_=acc,
                            func=AF.Relu,
                            bias=b1s[:, m : m + 1],
                            scale=1.0,
                        )

                # previous super-tile's output transposes (rx long since ready)
                if pend is not None:
                    emit_out(pend)
                    pend = None

                # phase C (layer 2 + fused layer 3): per m-slice, W2 streamed
                # once for both halves; h2 m-slice feeds accumulating zy/zx
                # matmuls. L3 matmuls lag one m so PE never waits on ACT.
                zy0 = zz_pool.tile([P, HALF], f32, tag="zy0", name="zy0")
                zx0 = zz_pool.tile([P, HALF], f32, tag="zx0", name="zx0")
                zy1 = zz_pool.tile([P, HALF], f32, tag="zy1", name="zy1")
                zx1 = zz_pool.tile([P, HALF], f32, tag="zx1", name="zx1")
                zzs = ((zy0, zx0), (zy1, zx1))
                h2q = []  # (m, h, h2m) awaiting L3 issue

                def issue_l3(upto_m):
                    while h2q and h2q[0][0] < upto_m:
                        im, ih, ih2 = h2q.pop(0)
                        zyh, zxh = zzs[ih]
                        nc.tensor.matmul(
                            zyh, w3ys[:, im, :], ih2,
                            start=(im == 0), stop=(im == K_H - 1),
                        )
                        nc.tensor.matmul(
                            zxh, w3xs[:, im, :], ih2,
                            start=(im == 0), stop=(im == K_H - 1),
                        )

                for m in range(K_H):
                    w2m = w2_pool.tile([P, K_H, P], f32r, tag="w2m", name=f"w2m{s}_{m}")
                    nc.sync.dma_start(
                        out=w2m,
                        in_=w2t_ext[m].rearrange("p (k c) -> p k c", k=K_H),
                    )
                    for h in range(2):
                        acc = mm_pool.tile([P, HALF], f32, tag="mm", name="acc")
                        for k in range(K_H):
                            nc.tensor.matmul(
                                acc,
                                w2m[:, k, :],
                                h1[:, k, h, :],
                                start=(k == 0),
                                stop=(k == K_H - 1),
                            )
                        h2m = h2_pool.tile([P, HALF], f32r, tag="h2m", name="h2m")
                        nc.scalar.activation(
                            out=h2m,
                            in_=acc,
                            func=AF.Relu,
                            bias=b2s[:, m : m + 1],
                            scale=1.0,
                        )
                        h2q.append((m, h, h2m))
                    issue_l3(m)
                issue_l3(K_H)

                # phase D (compute only; PE transposes deferred to next s):
                # atan2(ty, tx)/pi = Arctan(ty/tx)/pi + sign(ty)*(1-sign(tx))/2
                rxs = []
                for h in range(2):
                    zyh, zxh = zzs[h]
                    ty = epi_pool.tile([P, HALF], f32, tag="ty", name="ty")
                    nc.scalar.activation(out=ty, in_=zyh, func=AF.Tanh, bias=b3ys, scale=1.0)
                    tx = epi_pool.tile([P, HALF], f32, tag="tx", name="tx")
                    nc.scalar.activation(out=tx, in_=zxh, func=AF.Tanh, bias=b3xs, scale=1.0)
                    rx = epi_pool.tile([P, HALF], f32, tag="rx", name="rx")
                    nc.vector.reciprocal(out=rx, in_=tx)
                    nc.vector.tensor_mul(rx, ty, rx)            # q = ty/tx
                    sy = epi_pool.tile([P, HALF], f32, tag="sy", name="sy")
                    nc.scalar.activation(out=sy, in_=ty, func=AF.Sign)
                    sx = epi_pool.tile([P, HALF], f32, tag="sx", name="sx")
                    nc.scalar.activation(out=sx, in_=tx, func=AF.Sign)
                    nc.scalar.activation(out=tx, in_=rx, func=AF.Arctan)  # a (tx dead)
                    nc.vector.tensor_mul(sx, sy, sx)            # sy*sx
                    nc.vector.tensor_sub(sy, sy, sx)            # d = sy*(1-sx)
                    nc.vector.tensor_scalar(
                        out=rx, in0=tx, scalar1=INV_PI, scalar2=None, op0=ALU.mult
                    )
                    nc.vector.tensor_scalar(
                        out=sy, in0=sy, scalar1=0.5, scalar2=None, op0=ALU.mult
                    )
                    nc.vector.tensor_add(rx, rx, sy)            # resT
                    rxs.append(rx)
                pend = (s, rxs)

            emit_out(pend)

    return nc


def _round_f32r(a):
    """Round-to-nearest to f32r granularity (low 12 mantissa bits zeroed)."""
    a = np.ascontiguousarray(np.asarray(a, np.float32))
    b = a.view(np.uint32)
    b = ((b + 0x800) & np.uint32(0xFFFFF000)).astype(np.uint32)
    return b.view(np.float32)


def _tile_w(w, k_tiles):
    """[M, K] row-major -> [M/128, 128p, K] where [m, p, k*128+c] = w[m*128+c, k*128+p]."""
    m_tiles = w.shape[0] // P
    t = w.reshape(m_tiles, P, k_tiles, P).transpose(0, 3, 2, 1)
    return np.ascontiguousarray(t.reshape(m_tiles, P, k_tiles * P))


def _host_prep(states, bn_weight, bn_bias, w1, b1, w2, b2, w3, b3):
    w1t = _round_f32r(_tile_w(np.asarray(w1, np.float32), K_IN))
    w2t = _round_f32r(_tile_w(np.asarray(w2, np.float32), K_H))
    w3 = np.asarray(w3, np.float32)
    w3yt = _round_f32r(w3[0::2].T)   # [D_H, D_ACT]
    w3xt = _round_f32r(w3[1::2].T)
    b1r = np.ascontiguousarray(np.asarray(b1, np.float32).reshape(K_H, P).T)
    b2r = np.ascontiguousarray(np.asarray(b2, np.float32).reshape(K_H, P).T)
    b3 = np.asarray(b3, np.float32)
    b3y = np.ascontiguousarray(b3[0::2].reshape(P, 1))
    b3x = np.ascontiguousarray(b3[1::2].reshape(P, 1))
    bnw = np.ascontiguousarray(np.asarray(bn_weight, np.float32).reshape(K_IN, P).T)
    bnb = np.ascontiguousarray(np.asarray(bn_bias, np.float32).reshape(K_IN, P).T)
    shared = {
        "w1t": w1t, "w2t": w2t, "w3yt": w3yt, "w3xt": w3xt,
        "b1r": b1r, "b2r": b2r, "b3y": b3y, "b3x": b3x,
        "bnw": bnw, "bnb": bnb,
    }
    states = np.asarray(states, np.float32)
    in_maps = []
    for c in range(N_CORES):
        m = dict(shared)
        m["x"] = np.ascontiguousarray(states[c * B_CORE : (c + 1) * B_CORE])
        in_maps.append(m)
    return in_maps


def _get_ntff_hook():
    """Best-effort NTFF profiling hook (axon images without antenv.axon_hooks)."""
    try:
        from antenv.axon_hooks import get_axon_ntff_profile_hook

        return get_axon_ntff_profile_hook()
    except ImportError:
        pass
    try:
        from trn_agent_boot.trn_boot import _ntff_profile_via_ctypes

        return _ntff_profile_via_ctypes("/opt/axon/libaxon_pjrt.so")
    except Exception:
        return None


def _run(nc, in_maps, profile=True):
    """Run the SPMD kernel via PJRT; return (per-core results, exec_time_ns)."""
    import glob
    import os
    import tempfile

    from concourse import bass2jax

    hook = _get_ntff_hook() if profile else None
    if hook is None:
        return bass2jax.run_bass_via_pjrt(nc, in_maps, n_cores=N_CORES), None, None

    tmpdir = tempfile.mkdtemp(prefix="bass_ntff_")
    try:
        with hook(tmpdir, [0]):
            results = bass2jax.run_bass_via_pjrt(nc, in_maps, n_cores=N_CORES)
    except Exception as e:
        print(f"[kernel] NTFF hook failed ({type(e).__name__}: {e}); plain run")
        return bass2jax.run_bass_via_pjrt(nc, in_maps, n_cores=N_CORES), None, None

    exec_ns = None
    try:
        if glob.glob(os.path.join(tmpdir, "*_body*.ntff")):
            import gauge.profiler
            from concourse._compat import FishPath

            profile_obj = gauge.profiler.Profile(
                profile_path=FishPath(tmpdir),
                kernel_dev_mode=True,
                profile_on_exit=False,
                bass_kernel=nc.m,
                offline_processing=True,
                fname="*_body*",
            )
            prs = profile_obj.to_perfetto(model_index=(0,))
            if prs:
                exec_ns = max(p.exec_time_ns for p in prs if p.exec_time_ns)
    except Exception as e:
        print(f"[kernel] NTFF parse failed ({type(e).__name__}: {e})")
    return results, exec_ns, tmpdir


def kernel(**inputs):
    global LAST_EXEC_NS, LAST_TRACE_DIR
    if "nc" not in _CACHE:
        nc = _build_nc()
        if not nc.is_finalized():
            nc.finalize()
        _CACHE["nc"] = nc
    nc = _CACHE["nc"]

    in_maps = _host_prep(**inputs)
    results, exec_ns, trace_dir = _run(nc, in_maps)
    LAST_EXEC_NS = exec_ns
    LAST_TRACE_DIR = trace_dir
    out = np.concatenate([results[c]["out"] for c in range(N_CORES)], axis=0)
    return out.astype(np.float32)


# revision 6
# speedup vs baseline: 1.0804x; 1.0804x over previous
"""Trainium2 Bass kernel for the Actor MLP (BatchNorm -> 3-layer MLP -> atan2).

Data-parallel across 8 NeuronCores: batch sharded 8192 rows/core, weights
replicated. BatchNorm batch stats via per-shard bn_stats + 8KB AllReduce.

Matmuls run in float32r (fp32 with the mantissa rounded to ~11 bits): on
TRN2 this streams at the same 1 cycle/row as bf16 but with only ~2^-12
operand rounding, which keeps the atan2 branch-cut (ty ~ 0, tx < 0) sign
flips low (~294 of 8.4M elements, rel err 1.98e-2 vs bf16's 7.7e-2).
Weights are pre-rounded to f32r granularity on the host and DMA'd directly
into f32r tiles; activations are rounded by the ACT engine on PSUM
eviction.

Batch is processed in 1024-column super-tiles (two 512-col halves sharing
each streamed W1/W2 m-slice, halving weight HBM traffic vs per-512
streaming). Layer 3 (only 256 outputs) is fused into the layer-2 m-loop:
each h2 m-slice feeds two accumulating matmuls into resident zy/zx PSUM
banks, so h2 never materializes in SBUF and the freed space pays for the
1024-wide h1. The L3 matmuls for m-slice m-1 are issued after the L2
k-loops of m so the in-order PE queue never waits on an ACT eviction.
All activations stay transposed [feature, batch]; x is staged via 4MB
DMAs into slots aliased (by pool tag) onto the h1/xt buffers, which are
dead during the stats pass.
"""

import numpy as np

P = 128
B_CORE = 8192            # batch rows per core
SUP = 1024               # super-tile batch columns (2 halves of 512)
NSUP = B_CORE // SUP     # 8
HALF = 512
NJ = SUP // P            # 8 natural [128, D_IN] blocks per super-tile
D_IN = 1024
K_IN = D_IN // P         # 8
D_H = 2048
K_H = D_H // P           # 16
D_ACT = 128
BN_EPS = 1e-5
N_CORES = 8
INV_PI = float(1.0 / np.pi)

_CACHE = {}

LAST_EXEC_NS = None
LAST_TRACE_DIR = None


def _build_nc():
    import concourse.mybir as mybir
    import concourse.tile as tile
    from concourse import bacc
    from concourse.masks import make_identity

    f32 = mybir.dt.float32
    f32r = mybir.dt.float32r
    AF = mybir.ActivationFunctionType
    ALU = mybir.AluOpType

    nc = bacc.Bacc()

    x_ext = nc.declare_dram_parameter("x", [B_CORE, D_IN], f32, isOutput=False)
    # weights pre-tiled on host: [m_slice, partition, k*128] so each m-slice
    # DMA reads one fully-contiguous block per partition
    w1t_ext = nc.declare_dram_parameter("w1t", [K_H, P, D_IN], f32r, isOutput=False)
    w2t_ext = nc.declare_dram_parameter("w2t", [K_H, P, D_H], f32r, isOutput=False)
    w3yt_ext = nc.declare_dram_parameter("w3yt", [D_H, D_ACT], f32r, isOutput=False)
    w3xt_ext = nc.declare_dram_parameter("w3xt", [D_H, D_ACT], f32r, isOutput=False)
    b1_ext = nc.declare_dram_parameter("b1r", [P, K_H], f32, isOutput=False)
    b2_ext = nc.declare_dram_parameter("b2r", [P, K_H], f32, isOutput=False)
    b3y_ext = nc.declare_dram_parameter("b3y", [P, 1], f32, isOutput=False)
    b3x_ext = nc.declare_dram_parameter("b3x", [P, 1], f32, isOutput=False)
    bnw_ext = nc.declare_dram_parameter("bnw", [P, K_IN], f32, isOutput=False)
    bnb_ext = nc.declare_dram_parameter("bnb", [P, K_IN], f32, isOutput=False)
    out_ext = nc.declare_dram_parameter("out", [B_CORE, D_ACT], f32, isOutput=True)

    with tile.TileContext(nc) as tc:
        with (
            tc.tile_pool(name="singles", bufs=1) as singles,
            tc.tile_pool(name="small", bufs=1) as small,
            # tags h1 (64KB) / xt (32KB); the stats pass stages raw x in the
            # same slots (they're dead until the MLP pass)
            tc.tile_pool(name="acts", bufs=1) as acts,
            tc.tile_pool(name="xn", bufs=4) as xn_pool,
            tc.tile_pool(name="w1m", bufs=3) as w1_pool,
            tc.tile_pool(name="w2m", bufs=3) as w2_pool,
            tc.tile_pool(name="h2m", bufs=4) as h2_pool,
            tc.tile_pool(name="epi", bufs=1) as epi_pool,
            tc.tile_pool(name="res", bufs=2) as res_pool,
            tc.tile_pool(name="xps", bufs=2, space="PSUM") as xps_pool,
            tc.tile_pool(name="mm", bufs=2, space="PSUM") as mm_pool,
            tc.tile_pool(name="zz", bufs=1, space="PSUM") as zz_pool,
            tc.tile_pool(name="dram", bufs=1, space="DRAM") as dram_pool,
        ):
            # ---- constants ----
            ident = singles.tile([P, P], f32)
            make_identity(nc, ident)

            bnws = singles.tile([P, K_IN], f32)
            nc.sync.dma_start(out=bnws, in_=bnw_ext[:])
            bnbs = singles.tile([P, K_IN], f32)
            nc.sync.dma_start(out=bnbs, in_=bnb_ext[:])

            # ---- pass 1: batch stats (4MB staged loads -> transpose -> bn_stats) ----
            stats = small.tile([P, K_IN, 2 * NSUP, 6], f32)
            mv = small.tile([P, K_IN, 2], f32)
            # pk = [mean, E[x^2]] / N_CORES, packed for the AllReduce
            pk = small.tile([P, K_IN, 2], f32)
            for s in range(NSUP):
                stage = acts.tile(
                    [P, NJ, D_IN], f32,
                    tag=("h1" if s % 2 == 0 else "xt"), name=f"stage{s}",
                )
                nc.sync.dma_start(
                    out=stage,
                    in_=x_ext[s * SUP : (s + 1) * SUP, :].rearrange(
                        "(j p) c -> p j c", p=P
                    ),
                )
                for k in range(K_IN):
                    for h in range(2):
                        ps = xps_pool.tile([P, HALF], f32, tag="xps", name="ps")
                        for jj in range(4):
                            j = h * 4 + jj
                            nc.tensor.transpose(
                                ps[:, jj * P : (jj + 1) * P],
                                stage[:, j, k * P : (k + 1) * P],
                                ident,
                            )
                        nc.vector.bn_stats(out=stats[:, k, s * 2 + h, :], in_=ps)
                    if s == NSUP - 1:
                        # aggregate per k as soon as its last bn_stats lands
                        nc.vector.bn_aggr(out=mv[:, k, :], in_=stats[:, k, :, :])

            # MLP-pass constants, loaded behind the stats pass so the X stages
            # get the DMA queues at t=0
            w3ys = singles.tile([P, K_H, D_ACT], f32r)
            nc.sync.dma_start(out=w3ys, in_=w3yt_ext[:].rearrange("(k p) a -> p k a", p=P))
            w3xs = singles.tile([P, K_H, D_ACT], f32r)
            nc.sync.dma_start(out=w3xs, in_=w3xt_ext[:].rearrange("(k p) a -> p k a", p=P))
            b1s = singles.tile([P, K_H], f32)
            nc.sync.dma_start(out=b1s, in_=b1_ext[:])
            b2s = singles.tile([P, K_H], f32)
            nc.sync.dma_start(out=b2s, in_=b2_ext[:])
            b3ys = singles.tile([P, 1], f32)
            nc.sync.dma_start(out=b3ys, in_=b3y_ext[:])
            b3xs = singles.tile([P, 1], f32)
            nc.sync.dma_start(out=b3xs, in_=b3x_ext[:])

            # pack [mean, E[x^2]]/N_CORES for the AllReduce
            nc.vector.tensor_scalar_mul(pk[:, :, 0], mv[:, :, 0], 1.0 / N_CORES)
            nc.vector.tensor_mul(pk[:, :, 1], mv[:, :, 0], mv[:, :, 0])
            nc.vector.tensor_add(pk[:, :, 1], pk[:, :, 1], mv[:, :, 1])
            nc.vector.tensor_scalar_mul(pk[:, :, 1], pk[:, :, 1], 1.0 / N_CORES)

            cc_in = dram_pool.tile([P, K_IN, 2], f32)
            cc_out = dram_pool.tile([P, K_IN, 2], f32)
            # gpsimd's queue, so this 8KB doesn't sit behind weight prefetch MBs
            nc.gpsimd.dma_start(out=cc_in, in_=pk)
            nc.gpsimd.collective_compute(
                "AllReduce",
                ALU.add,
                replica_groups=[list(range(N_CORES))],
                ins=[cc_in.opt()],
                outs=[cc_out.opt()],
            )
            g = small.tile([P, K_IN, 2], f32)
            nc.gpsimd.dma_start(out=g, in_=cc_out)

            # global mean / var -> per-feature scale & shift
            gm = g[:, :, 0]
            var = small.tile([P, K_IN], f32)
            nc.vector.tensor_copy(out=var, in_=g[:, :, 1])
            gm2 = small.tile([P, K_IN], f32)
            nc.vector.tensor_mul(gm2, gm, gm)
            nc.vector.tensor_sub(var, var, gm2)
            eps_t = small.tile([P, 1], f32)
            nc.vector.memset(eps_t, BN_EPS)
            sq = small.tile([P, K_IN], f32)
            nc.scalar.activation(out=sq, in_=var, func=AF.Sqrt, bias=eps_t, scale=1.0)
            rstd = small.tile([P, K_IN], f32)
            nc.vector.reciprocal(out=rstd, in_=sq)
            scale = small.tile([P, K_IN], f32)
            nc.vector.tensor_mul(scale, bnws, rstd)
            shift = small.tile([P, K_IN], f32)
            nc.vector.tensor_mul(shift, gm, scale)
            nc.vector.tensor_sub(shift, bnbs, shift)

            # ---- pass 2: normalize + 3-layer MLP + atan2 epilogue ----
            pend = None  # deferred output-transpose work: (s, [rx_h0, rx_h1])

            def emit_out(pd):
                ps_, rxs = pd
                for h in range(2):
                    rp = xps_pool.tile([P, 4, P], f32, tag="xps", name="rp")
                    for jj in range(4):
                        nc.tensor.transpose(
                            rp[:, jj, :], rxs[h][:, jj * P : (jj + 1) * P], ident
                        )
                    rn = res_pool.tile([P, 4, P], f32, tag="rn", name="rn")
                    nc.scalar.activation(out=rn, in_=rp, func=AF.Copy)
                    r0 = ps_ * SUP + h * HALF
                    nc.sync.dma_start(
                        out=out_ext[r0 : r0 + HALF, :].rearrange(
                            "(j p) a -> p j a", p=P
                        ),
                        in_=rn,
                    )

            def load_xn(s):
                xn = []
                for jb in range(4):
                    xnt = xn_pool.tile([P, 2, D_IN], f32, tag="xn", name=f"xn{s}_{jb}")
                    r0 = s * SUP + jb * 256
                    nc.sync.dma_start(
                        out=xnt,
                        in_=x_ext[r0 : r0 + 256, :].rearrange("(j p) c -> p j c", p=P),
                    )
                    xn.append(xnt)
                return xn

            next_xn = load_xn(0)
            for s in range(NSUP):
                xn = next_xn

                # phase A: transpose + fused (x*scale + shift) normalize -> f32r
                xt = acts.tile([P, K_IN, 2, HALF], f32r, tag="xt", name=f"xt{s}")
                for h in range(2):
                    for k in range(K_IN):
                        ps = xps_pool.tile([P, HALF], f32, tag="xps", name="ps")
                        for jj in range(4):
                            j = h * 4 + jj
                            nc.tensor.transpose(
                                ps[:, jj * P : (jj + 1) * P],
                                xn[j // 2][:, j % 2, k * P : (k + 1) * P],
                                ident,
                            )
                        nc.scalar.activation(
                            out=xt[:, k, h, :],
                            in_=ps,
                            func=AF.Identity,
                            bias=shift[:, k : k + 1],
                            scale=scale[:, k : k + 1],
                        )

                # phase B (layer 1): h1T = relu(W1 @ xnT + b1); W1 m-slice
                # streamed once, used by both halves
                h1 = acts.tile([P, K_H, 2, HALF], f32r, tag="h1", name=f"h1_{s}")
                for m in range(K_H):
                    w1m = w1_pool.tile([P, K_IN, P], f32r, tag="w1m", name=f"w1m{s}_{m}")
                    nc.sync.dma_start(
                        out=w1m,
                        in_=w1t_ext[m].rearrange("p (k c) -> p k c", k=K_IN),
                    )
                    for h in range(2):
                        acc = mm_pool.tile([P, HALF], f32, tag="mm", name="acc")
                        for k in range(K_IN):
                            nc.tensor.matmul(
                                acc,
                                w1m[:, k, :],
                                xt[:, k, h, :],
                                start=(k == 0),
                                stop=(k == K_IN - 1),
                            )
                        nc.scalar.activation(
                            out=h1[:, m, h, :],
                            in_=acc,
                            func=AF.Relu,
                            bias=b1s[:, m : m + 1],
                            scale=1.0,
                        )

                # previous super-tile's output transposes (rx long since ready)
                if pend is not None:
                    emit_out(pend)
                    pend = None

                # phase C (layer 2 + fused layer 3): per m-slice, W2 streamed
                # once for both halves; h2 m-slice feeds accumulating zy/zx
                # matmuls. L3 matmuls lag one m so PE never waits on ACT.
                zy0 = zz_pool.tile([P, HALF], f32, tag="zy0", name="zy0")
                zx0 = zz_pool.tile([P, HALF], f32, tag="zx0", name="zx0")
                zy1 = zz_pool.tile([P, HALF], f32, tag="zy1", name="zy1")
                zx1 = zz_pool.tile([P, HALF], f32, tag="zx1", name="zx1")
                zzs = ((zy0, zx0), (zy1, zx1))
                h2q = []  # (m, h, h2m) awaiting L3 issue

                def issue_l3(upto_m):
                    while h2q and h2q[0][0] < upto_m:
                        im, ih, ih2 = h2q.pop(0)
                        zyh, zxh = zzs[ih]
                        nc.tensor.matmul(
                            zyh, w3ys[:, im, :], ih2,
                            start=(im == 0), stop=(im == K_H - 1),
                        )
                        nc.tensor.matmul(
                            zxh, w3xs[:, im, :], ih2,
                            start=(im == 0), stop=(im == K_H - 1),
                        )

                for m in range(K_H):
                    w2m = w2_pool.tile([P, K_H, P], f32r, tag="w2m", name=f"w2m{s}_{m}")
                    nc.sync.dma_start(
                        out=w2m,
                        in_=w2t_ext[m].rearrange("p (k c) -> p k c", k=K_H),
                    )
                    if m == 2 and s + 1 < NSUP:
                        next_xn = load_xn(s + 1)
                    for h in range(2):
                        acc = mm_pool.tile([P, HALF], f32, tag="mm", name="acc")
                        for k in range(K_H):
                            nc.tensor.matmul(
                                acc,
                                w2m[:, k, :],
                                h1[:, k, h, :],
                                start=(k == 0),
                                stop=(k == K_H - 1),
                            )
                        h2m = h2_pool.tile([P, HALF], f32r, tag="h2m", name="h2m")
                        nc.scalar.activation(
                            out=h2m,
                            in_=acc,
                            func=AF.Relu,
                            bias=b2s[:, m : m + 1],
                            scale=1.0,
                        )
                        h2q.append((m, h, h2m))
                    issue_l3(m)
                issue_l3(K_H)

                # phase D (compute only; PE transposes deferred to next s):
                # atan2(ty, tx)/pi = Arctan(ty/tx)/pi + sign(ty)*(1-sign(tx))/2
                rxs = []
                for h in range(2):
                    zyh, zxh = zzs[h]
                    ty = epi_pool.tile([P, HALF], f32, tag="ty", name="ty")
                    nc.scalar.activation(out=ty, in_=zyh, func=AF.Tanh, bias=b3ys, scale=1.0)
                    tx = epi_pool.tile([P, HALF], f32, tag="tx", name="tx")
                    nc.scalar.activation(out=tx, in_=zxh, func=AF.Tanh, bias=b3xs, scale=1.0)
                    rx = epi_pool.tile([P, HALF], f32, tag="rx", name="rx")
                    nc.vector.reciprocal(out=rx, in_=tx)
                    nc.vector.tensor_mul(rx, ty, rx)            # q = ty/tx
                    sy = epi_pool.tile([P, HALF], f32, tag="sy", name="sy")
                    nc.scalar.activation(out=sy, in_=ty, func=AF.Sign)
                    sx = epi_pool.tile([P, HALF], f32, tag="sx", name="sx")
                    nc.scalar.activation(out=sx, in_=tx, func=AF.Sign)
                    nc.scalar.activation(out=tx, in_=rx, func=AF.Arctan)  # a (tx dead)
                    nc.vector.tensor_mul(sx, sy, sx)            # sy*sx
                    nc.vector.tensor_sub(sy, sy, sx)            # d = sy*(1-sx)
                    nc.vector.tensor_scalar(
                        out=rx, in0=tx, scalar1=INV_PI, scalar2=None, op0=ALU.mult
                    )
                    nc.vector.tensor_scalar(
                        out=sy, in0=sy, scalar1=0.5, scalar2=None, op0=ALU.mult
                    )
                    nc.vector.tensor_add(rx, rx, sy)            # resT
                    rxs.append(rx)
                pend = (s, rxs)

            emit_out(pend)

    return nc


def _round_f32r(a):
    """Round-to-nearest to f32r granularity (low 12 mantissa bits zeroed)."""
    a = np.ascontiguousarray(np.asarray(a, np.float32))
    b = a.view(np.uint32)
    b = ((b + 0x800) & np.uint32(0xFFFFF000)).astype(np.uint32)
    return b.view(np.float32)


def _tile_w(w, k_tiles):
    """[M, K] row-major -> [M/128, 128p, K] where [m, p, k*128+c] = w[m*128+c, k*128+p]."""
    m_tiles = w.shape[0] // P
    t = w.reshape(m_tiles, P, k_tiles, P).transpose(0, 3, 2, 1)
    return np.ascontiguousarray(t.reshape(m_tiles, P, k_tiles * P))


def _host_prep(states, bn_weight, bn_bias, w1, b1, w2, b2, w3, b3):
    w1t = _round_f32r(_tile_w(np.asarray(w1, np.float32), K_IN))
    w2t = _round_f32r(_tile_w(np.asarray(w2, np.float32), K_H))
    w3 = np.asarray(w3, np.float32)
    w3yt = _round_f32r(w3[0::2].T)   # [D_H, D_ACT]
    w3xt = _round_f32r(w3[1::2].T)
    b1r = np.ascontiguousarray(np.asarray(b1, np.float32).reshape(K_H, P).T)
    b2r = np.ascontiguousarray(np.asarray(b2, np.float32).reshape(K_H, P).T)
    b3 = np.asarray(b3, np.float32)
    b3y = np.ascontiguousarray(b3[0::2].reshape(P, 1))
    b3x = np.ascontiguousarray(b3[1::2].reshape(P, 1))
    bnw = np.ascontiguousarray(np.asarray(bn_weight, np.float32).reshape(K_IN, P).T)
    bnb = np.ascontiguousarray(np.asarray(bn_bias, np.float32).reshape(K_IN, P).T)
    shared = {
        "w1t": w1t, "w2t": w2t, "w3yt": w3yt, "w3xt": w3xt,
        "b1r": b1r, "b2r": b2r, "b3y": b3y, "b3x": b3x,
        "bnw": bnw, "bnb": bnb,
    }
    states = np.asarray(states, np.float32)
    in_maps = []
    for c in range(N_CORES):
        m = dict(shared)
        m["x"] = np.ascontiguousarray(states[c * B_CORE : (c + 1) * B_CORE])
        in_maps.append(m)
    return in_maps


def _get_ntff_hook():
    """Best-effort NTFF profiling hook (axon images without antenv.axon_hooks)."""
    try:
        from antenv.axon_hooks import get_axon_ntff_profile_hook

        return get_axon_ntff_profile_hook()
    except ImportError:
        pass
    try:
        from trn_agent_boot.trn_boot import _ntff_profile_via_ctypes

        return _ntff_profile_via_ctypes("/opt/axon/libaxon_pjrt.so")
    except Exception:
        return None


def _run(nc, in_maps, profile=True):
    """Run the SPMD kernel via PJRT; return (per-core results, exec_time_ns)."""
    import glob
    import os
    import tempfile

    from concourse import bass2jax

    hook = _get_ntff_hook() if profile else None
    if hook is None:
        return bass2jax.run_bass_via_pjrt(nc, in_maps, n_cores=N_CORES), None, None

    tmpdir = tempfile.mkdtemp(prefix="bass_ntff_")
    try:
        with hook(tmpdir, [0]):
            results = bass2jax.run_bass_via_pjrt(nc, in_maps, n_cores=N_CORES)
    except Exception as e:
        print(f"[kernel] NTFF hook failed ({type(e).__name__}: {e}); plain run")
        return bass2jax.run_bass_via_pjrt(nc, in_maps, n_cores=N_CORES), None, None

    exec_ns = None
    try:
        if glob.glob(os.path.join(tmpdir, "*_body*.ntff")):
            import gauge.profiler
            from concourse._compat import FishPath

            profile_obj = gauge.profiler.Profile(
                profile_path=FishPath(tmpdir),
                kernel_dev_mode=True,
                profile_on_exit=False,
                bass_kernel=nc.m,
                offline_processing=True,
                fname="*_body*",
            )
            prs = profile_obj.to_perfetto(model_index=(0,))
            if prs:
                exec_ns = max(p.exec_time_ns for p in prs if p.exec_time_ns)
    except Exception as e:
        print(f"[kernel] NTFF parse failed ({type(e).__name__}: {e})")
    return results, exec_ns, tmpdir


def kernel(**inputs):
    global LAST_EXEC_NS, LAST_TRACE_DIR
    if "nc" not in _CACHE:
        nc = _build_nc()
        if not nc.is_finalized():
            nc.finalize()
        _CACHE["nc"] = nc
    nc = _CACHE["nc"]

    in_maps = _host_prep(**inputs)
    results, exec_ns, trace_dir = _run(nc, in_maps)
    LAST_EXEC_NS = exec_ns
    LAST_TRACE_DIR = trace_dir
    out = np.concatenate([results[c]["out"] for c in range(N_CORES)], axis=0)
    return out.astype(np.float32)
